# revision 9
# baseline (speedup 1.0000x reference)
"""Trainium2 Bass kernel for pairwise Tang-Toennies dispersion energy.

Problem: for P=3.2M random atom pairs over N=100k atoms in a periodic box,
    ene[p] = -(c6[ti,tj] * f6(b[ti,tj]*r)) / r^6   if r <= cutoff else 0
with r the minimum-image distance and f6 the 6th-order Tang-Toennies damper.

Only ~2% of random pairs fall inside the 10A cutoff, so the kernel is split
into a cheap dense screen and an exact sparse evaluation:

  host:     normalize coords to box units (x/L, an O(N) table prep; the
            i-side planes are negated so the device subtract is an add) and
            gather per-pair SoA planes jx..jz / ix..iz (pure index routing).
  kernel A: (dense, 3.2M slots) minimum-image r^2 per axis via
            m = min(|d|, 1-|d|) (exactly equal to the reference's round()
            form for |d| < 1), then a u8 in-cutoff mask.  Abs/Square run on
            the Act engine from a single activation-table set, so the hot
            loop has no table switches.
  host:     compacts the device-produced mask (np.flatnonzero + index
            gathers -- no host float math decides anything).
  kernel B: (sparse, ~62k slots) full energy: r^2 recomputed identically,
            r and r^-6 via Ln/Exp, Tang-Toennies f6 via an Estrin-form
            polynomial, exact series fallback for r^2<=0.4 where the
            1 - e^-u*poly form is pure f32 cancellation noise.
  host:     scatters the survivor energies into the zero-filled output.

Work is sharded by slots: every core gets the same slot count for both
kernels, so the 8 cores are perfectly balanced.
"""

import contextlib

import numpy as np

import concourse.bacc as bacc
import concourse.bass as bass
import concourse.mybir as mybir
from concourse.tile import TileContext
from concourse.bass_utils import run_bass_kernel_spmd

F32 = mybir.dt.float32
U8 = mybir.dt.uint8
U16 = mybir.dt.uint16
AF = mybir.ActivationFunctionType
OP = mybir.AluOpType

N_CORES = 8

# kernel A (dense screen) tiling: slots/core = 128 * KA * TA
KA = 1042
TA = 3
SLOTS_A = 128 * KA * TA          # 400,128 per core

# kernel B (sparse energy) tiling
KB = 80
TB = 1
SLOTS_B = 128 * KB * TB          # 10,240 per core (81,920 total capacity)

AXES = ("x", "y", "z")


def _geometry(nc, pw, w, K, Ls, nchunk, period=1.0, sub=False):
    """Fused 3-axis minimum-image: L^2 m^2 summed into r^2.

    `w` is a packed f32 tile [128, nchunk*K] whose chunks are
    [jx, ix, jy, iy, jz, iz, ...].  With sub=False the i chunks are
    NEGATED host-side (d = j + i_neg); with sub=True d = j - i.  Coords
    are in units of `period` per box length (1.0 for normalized f32,
    65536.0 for u16-quantized), folded into the min-image reflection and
    the Square scale.  All per-axis steps run as single strided ops over
    [128, 3, K].
    """
    jv = bass.AP(w.tensor, 0, [w[:].ap[0], [2 * K, 3], [1, K]])
    iv = bass.AP(w.tensor, K, [w[:].ap[0], [2 * K, 3], [1, K]])
    d = pw.tile([128, 3 * K], F32, tag="d", name="d")
    d3 = d[:].rearrange("p (a k) -> p a k", a=3)
    if sub:
        nc.vector.tensor_tensor(out=d3, in0=jv, in1=iv, op=OP.subtract)
    else:
        nc.gpsimd.tensor_tensor(out=d3, in0=jv, in1=iv, op=OP.add)
    a1 = pw.tile([128, 3 * K], F32, tag="a1", name="a1")
    nc.scalar.activation(a1[:], d[:], AF.Abs)
    w1 = pw.tile([128, 3 * K], F32, tag="w1", name="w1")
    nc.gpsimd.tensor_scalar(out=w1[:], in0=a1[:], scalar1=-1.0,
                            scalar2=float(period), op0=OP.mult, op1=OP.add)
    m = pw.tile([128, 3 * K], F32, tag="m", name="m")
    nc.vector.tensor_tensor(out=m[:], in0=a1[:], in1=w1[:], op=OP.min)
    sq = pw.tile([128, 3 * K], F32, tag="sq", name="sq")
    m3 = m[:].rearrange("p (a k) -> p a k", a=3)
    sq3 = sq[:].rearrange("p (a k) -> p a k", a=3)
    if Ls[0] == Ls[1] == Ls[2]:
        nc.scalar.activation(sq[:], m[:], AF.Square,
                             scale=float(Ls[0]) / float(period))
    else:
        for ci in range(3):
            nc.scalar.activation(sq3[:, ci, :], m3[:, ci, :], AF.Square,
                                 scale=float(Ls[ci]) / float(period))
    r2 = pw.tile([128, K], F32, tag="r2", name="r2")
    nc.vector.tensor_tensor(out=r2[:], in0=sq3[:, 0, :], in1=sq3[:, 1, :],
                            op=OP.add)
    nc.gpsimd.tensor_tensor(out=r2[:], in0=r2[:], in1=sq3[:, 2, :], op=OP.add)
    return r2


def build_a(Ls, cutoff, reps=1, t_limit=None):
    """Dense screen: per-slot minimum-image r^2 -> u8 (r2 <= cutoff^2)."""
    nc = bacc.Bacc(trn_type="TRN2", target_bir_lowering=False)
    pk_d = nc.dram_tensor("pk", [TA * 128 * 6 * KA], U16, kind="ExternalInput")
    mask_d = nc.dram_tensor("mask", [SLOTS_A], U8, kind="ExternalOutput")
    c2 = float(np.float32(cutoff) ** 2)
    T = TA if t_limit is None else min(TA, t_limit)

    with TileContext(nc) as tc:
        rep_ctx = tc.For_i(0, reps, 1) if reps > 1 else contextlib.nullcontext()
        with tc.tile_pool(name="io", bufs=2) as pio, \
             tc.tile_pool(name="work", bufs=2) as pw, rep_ctx:
            for t in range(T):
                wq = pio.tile([128, 6 * KA], U16, tag="wq", name="wq")
                nc.sync.dma_start(
                    out=wq[:],
                    in_=pk_d[bass.ts(t, 128 * 6 * KA)].rearrange(
                        "(p x) -> p x", x=6 * KA))
                w = pw.tile([128, 6 * KA], F32, tag="w", name="w")
                jq = bass.AP(wq.tensor, 0, [wq[:].ap[0], [2 * KA, 3], [1, KA]])
                iq = bass.AP(wq.tensor, KA, [wq[:].ap[0], [2 * KA, 3], [1, KA]])
                jf = bass.AP(w.tensor, 0, [w[:].ap[0], [2 * KA, 3], [1, KA]])
                if_ = bass.AP(w.tensor, KA, [w[:].ap[0], [2 * KA, 3], [1, KA]])
                nc.gpsimd.tensor_copy(jf, jq)
                nc.scalar.activation(if_, iq, AF.Copy)
                r2 = _geometry(nc, pw, w, KA, Ls, 6, period=65536.0, sub=True)
                mk = pio.tile([128, KA], U8, tag="mk")
                nc.vector.tensor_scalar(out=mk[:], in0=r2[:], scalar1=c2,
                                        scalar2=None, op0=OP.is_le)
                nc.sync.dma_start(
                    out=mask_d[bass.ts(t, 128 * KA)].rearrange(
                        "(p x) -> p x", x=KA),
                    in_=mk[:])
    nc.compile()
    return nc


def build_b(Ls, cutoff, reps=1):
    """Sparse exact energy on compacted in-cutoff slots.

    Inputs: negated-i coord planes, cp = -c6[ti,tj], bp = b[ti,tj].
    """
    nc = bacc.Bacc(trn_type="TRN2", target_bir_lowering=False)
    pk_d = nc.dram_tensor("pk", [TB * 128 * 8 * KB], F32, kind="ExternalInput")
    ene_d = nc.dram_tensor("ene", [SLOTS_B], F32, kind="ExternalOutput")
    c2 = float(np.float32(cutoff) ** 2)
    K = KB

    with TileContext(nc) as tc:
        rep_ctx = tc.For_i(0, reps, 1) if reps > 1 else contextlib.nullcontext()
        with tc.tile_pool(name="io", bufs=2) as pio, \
             tc.tile_pool(name="work", bufs=2) as pw, rep_ctx:
            for t in range(TB):
                w = pio.tile([128, 8 * K], F32, tag="w", name="w")
                nc.sync.dma_start(
                    out=w[:],
                    in_=pk_d[bass.ts(t, 128 * 8 * K)].rearrange(
                        "(p x) -> p x", x=8 * K))
                wv = w[:].rearrange("p (c k) -> p c k", c=8)
                cp = wv[:, 6, :]
                bp = wv[:, 7, :]

                # ---- geometry: identical arithmetic to kernel A ----
                r2 = _geometry(nc, pw, w, K, Ls, 8)

                # ---- r, r^-6 via one Ln + two Exp ----
                lr2 = pw.tile([128, K], F32, tag="lr2")
                nc.scalar.activation(lr2[:], r2[:], AF.Ln)
                rr = pw.tile([128, K], F32, tag="rr")
                nc.scalar.activation(rr[:], lr2[:], AF.Exp, scale=0.5)
                ir6 = pw.tile([128, K], F32, tag="ir6")
                nc.scalar.activation(ir6[:], lr2[:], AF.Exp, scale=-3.0)

                u = pw.tile([128, K], F32, tag="u")
                nc.vector.tensor_tensor(out=u[:], in0=bp, in1=rr[:],
                                        op=OP.mult)
                em = pw.tile([128, K], F32, tag="em")
                nc.scalar.activation(em[:], u[:], AF.Exp, scale=-1.0)
                w = pw.tile([128, K], F32, tag="w")
                nc.scalar.activation(w[:], u[:], AF.Square)

                # ---- poly(u) = sum_0^6 u^k/k!, Estrin form:
                # (1+u) + w*(1/2 + u/6) + w^2*((1/24 + u/120) + w/720)
                a_ = pw.tile([128, K], F32, tag="a_")
                nc.vector.tensor_scalar(out=a_[:], in0=u[:], scalar1=1.0 / 6.0,
                                        scalar2=0.5, op0=OP.mult, op1=OP.add)
                bq = pw.tile([128, K], F32, tag="bq")
                nc.gpsimd.tensor_scalar(out=bq[:], in0=u[:], scalar1=1.0 / 120.0,
                                        scalar2=1.0 / 24.0, op0=OP.mult,
                                        op1=OP.add)
                d2 = pw.tile([128, K], F32, tag="d2")
                nc.vector.scalar_tensor_tensor(out=d2[:], in0=w[:],
                                               scalar=1.0 / 720.0, in1=bq[:],
                                               op0=OP.mult, op1=OP.add)
                t1 = pw.tile([128, K], F32, tag="t1")
                nc.gpsimd.tensor_scalar(out=t1[:], in0=u[:], scalar1=1.0,
                                        scalar2=None, op0=OP.add)
                g = pw.tile([128, K], F32, tag="g")
                nc.vector.tensor_tensor(out=g[:], in0=w[:], in1=d2[:],
                                        op=OP.mult)
                nc.gpsimd.tensor_tensor(out=g[:], in0=g[:], in1=a_[:],
                                        op=OP.add)
                nc.vector.tensor_tensor(out=g[:], in0=w[:], in1=g[:],
                                        op=OP.mult)
                p = pw.tile([128, K], F32, tag="p")
                nc.gpsimd.tensor_tensor(out=p[:], in0=g[:], in1=t1[:],
                                        op=OP.add)

                # ---- ene = B'*poly - A' with A' = c6/r^6.  cp = -c6, so
                # An = -A', Bn = An*em = -B', ene = (-1*Bn)*poly + An. ----
                An = pw.tile([128, K], F32, tag="An")
                nc.vector.tensor_tensor(out=An[:], in0=cp, in1=ir6[:],
                                        op=OP.mult)
                Bn = pw.tile([128, K], F32, tag="Bn")
                nc.gpsimd.tensor_tensor(out=Bn[:], in0=An[:], in1=em[:],
                                        op=OP.mult)
                ene = pw.tile([128, K], F32, tag="ene")
                nc.vector.scalar_tensor_tensor(out=ene[:], in0=Bn[:],
                                               scalar=-1.0, in1=p[:],
                                               op0=OP.mult, op1=OP.mult)
                nc.gpsimd.tensor_tensor(out=ene[:], in0=ene[:], in1=An[:],
                                        op=OP.add)

                # ---- small-u exact series: f6 = em*u^7/5040*(1+u/8+u^2/72)
                # (the direct 1-em*poly form is f32 cancellation noise there;
                # es = (Bn/5040)*u7*S = -B'/5040*u7*S, correctly negative)
                u3 = pw.tile([128, K], F32, tag="u3")
                nc.vector.tensor_tensor(out=u3[:], in0=u[:], in1=w[:],
                                        op=OP.mult)
                u6 = pw.tile([128, K], F32, tag="u6")
                nc.scalar.activation(u6[:], u3[:], AF.Square)
                u7 = pw.tile([128, K], F32, tag="u7")
                nc.gpsimd.tensor_tensor(out=u7[:], in0=u6[:], in1=u[:],
                                        op=OP.mult)
                s1u = pw.tile([128, K], F32, tag="s1u")
                nc.vector.tensor_scalar(out=s1u[:], in0=u[:], scalar1=1.0 / 8.0,
                                        scalar2=1.0, op0=OP.mult, op1=OP.add)
                S = pw.tile([128, K], F32, tag="S")
                nc.vector.scalar_tensor_tensor(out=S[:], in0=w[:],
                                               scalar=1.0 / 72.0, in1=s1u[:],
                                               op0=OP.mult, op1=OP.add)
                es = pw.tile([128, K], F32, tag="es")
                nc.gpsimd.tensor_tensor(out=es[:], in0=u7[:], in1=S[:],
                                        op=OP.mult)
                nc.vector.scalar_tensor_tensor(out=es[:], in0=Bn[:],
                                               scalar=1.0 / 5040.0, in1=es[:],
                                               op0=OP.mult, op1=OP.mult)
                mu = pw.tile([128, K], U8, tag="mu")
                nc.vector.tensor_scalar(out=mu[:], in0=r2[:], scalar1=0.4,
                                        scalar2=None, op0=OP.is_le)
                nc.vector.select(out=ene[:], mask=mu[:], on_true=es[:],
                                 on_false=ene[:])

                # ---- cutoff (identical compare to kernel A's mask) ----
                eout = pio.tile([128, K], F32, tag="eout")
                nc.vector.scalar_tensor_tensor(out=eout[:], in0=r2[:],
                                               scalar=c2, in1=ene[:],
                                               op0=OP.is_le, op1=OP.mult)
                nc.sync.dma_start(
                    out=ene_d[bass.ts(t, 128 * K)].rearrange(
                        "(p x) -> p x", x=K),
                    in_=eout[:])
    nc.compile()
    return nc


_NC_CACHE = {}


def _get_nc(builder, key, *args, **kw):
    if key not in _NC_CACHE:
        _NC_CACHE[key] = builder(*args, **kw)
    return _NC_CACHE[key]


def _host_reference(coords, pairs, box, c6, b, cutoff, atom_types):
    # numpy fallback for non-orthorhombic boxes (not hit by the real inputs)
    dr = coords[pairs[:, 1]] - coords[pairs[:, 0]]
    inv_box = np.linalg.inv(box)
    dr = dr - np.round(dr @ inv_box) @ box
    r = np.sqrt((dr * dr).sum(1))
    ti = atom_types[pairs[:, 0]]
    tj = atom_types[pairs[:, 1]]
    u = b[ti, tj] * r
    poly = 1.0 + u * (1.0 + u / 2.0 * (1.0 + u / 3.0 * (1.0 + u / 4.0 *
                     (1.0 + u / 5.0 * (1.0 + u / 6.0)))))
    f6 = 1.0 - np.exp(-u) * poly
    ene = -(c6[ti, tj] * f6) / r ** 6
    return np.where(r <= cutoff, ene, 0.0).astype(np.float32)


def _quant_coords(coords_n):
    """u16 fixed-point box-fraction coords (exact mod-2^16 wrap)."""
    q = np.round(coords_n.astype(np.float64) * 65536.0).astype(np.int64)
    return (q % 65536).astype(np.uint16)


def _plane_sets_a(coords_n, pi, pj):
    """Per-core kernel-A inputs: one packed u16 tensor per core with
    per-tile chunk layout [jx, ix, jy, iy, jz, iz]."""
    P = pi.shape[0]
    cq = _quant_coords(coords_n)
    total = N_CORES * SLOTS_A
    pk = np.empty((6, total), np.uint16)
    for ci in range(3):
        pk[2 * ci, :P] = cq[pj, ci]
        pk[2 * ci, P:] = 16384
        pk[2 * ci + 1, :P] = cq[pi, ci]
        pk[2 * ci + 1, P:] = 0
    # [6, cores, T, 128, K] -> [cores, T, 128, 6, K] contiguous
    pk = pk.reshape(6, N_CORES, TA, 128, KA).transpose(1, 2, 3, 0, 4)
    pk = np.ascontiguousarray(pk).reshape(N_CORES, -1)
    return [{"pk": pk[c]} for c in range(N_CORES)]


def _bufs_b(coords_n, c6, b, pi, pj, ti, tj, sl):
    """Kernel B packed input (chunks [jx,ix,jy,iy,jz,iz,cp,bp]) for one
    chunk of survivor indices `sl`.  cp = -c6, i chunks negated."""
    cap = N_CORES * SLOTS_B
    n = sl.shape[0]
    pk = np.empty((8, cap), np.float32)
    for ci in range(3):
        pk[2 * ci, n:] = 0.25
        pk[2 * ci, :n] = coords_n[pj[sl], ci]
        pk[2 * ci + 1, n:] = 0.0
        pk[2 * ci + 1, :n] = -coords_n[pi[sl], ci]
    pk[6, n:] = 0.0
    pk[6, :n] = -c6[ti, tj]
    pk[7, n:] = 1.0
    pk[7, :n] = b[ti, tj]
    pk = pk.reshape(8, N_CORES, TB, 128, KB).transpose(1, 2, 3, 0, 4)
    pk = np.ascontiguousarray(pk).reshape(N_CORES, -1)
    return {"pk": pk}


def kernel(coords, pairs, box, c6, b, cutoff, atom_types):
    coords = np.asarray(coords, np.float32)
    pairs = np.asarray(pairs)
    box = np.asarray(box, np.float32)
    c6 = np.asarray(c6, np.float32)
    b = np.asarray(b, np.float32)
    atom_types = np.asarray(atom_types).astype(np.int64)
    cutoff = float(np.asarray(cutoff))

    offdiag = box - np.diag(np.diag(box))
    if np.any(offdiag != 0.0) or pairs.shape[0] > N_CORES * SLOTS_A:
        return _host_reference(coords, pairs, box, c6, b, cutoff, atom_types)
    Ls = tuple(float(box[i, i]) for i in range(3))

    P = pairs.shape[0]
    pi = np.ascontiguousarray(pairs[:, 0]).astype(np.int64)
    pj = np.ascontiguousarray(pairs[:, 1]).astype(np.int64)
    coords_n = coords / np.asarray(Ls, np.float32)[None, :]

    # ---- kernel A: dense in-cutoff screen ----
    nc_a = _get_nc(build_a, ("a", Ls, round(cutoff, 6)), Ls, cutoff)
    in_a = _plane_sets_a(coords_n, pi, pj)
    res_a = run_bass_kernel_spmd(nc_a, in_a, core_ids=list(range(N_CORES)))
    mask = np.concatenate([res_a.results[c]["mask"] for c in range(N_CORES)])

    # ---- host: compact by the device-produced mask (index routing only) ----
    idx = np.flatnonzero(mask[:P])

    # ---- kernel B: exact energies for survivors (chunked if ever needed) ----
    nc_b = _get_nc(build_b, ("b", Ls, round(cutoff, 6)), Ls, cutoff)
    cap = N_CORES * SLOTS_B
    ene_s = np.empty(idx.shape[0], np.float32)
    for lo in range(0, max(idx.shape[0], 1), cap):
        sl = idx[lo:lo + cap]
        ti = atom_types[pi[sl]]
        tj = atom_types[pj[sl]]
        buf = _bufs_b(coords_n, c6, b, pi, pj, ti, tj, sl)
        in_b = [{"pk": buf["pk"][c]} for c in range(N_CORES)]
        res_b = run_bass_kernel_spmd(nc_b, in_b, core_ids=list(range(N_CORES)))
        ene_full = np.concatenate([res_b.results[c]["ene"]
                                   for c in range(N_CORES)])
        ene_s[lo:lo + sl.shape[0]] = ene_full[:sl.shape[0]]

    out = np.zeros(P, np.float32)
    out[idx] = ene_s
    return out


# revision 11
# speedup vs baseline: 1.5948x; 1.5948x over previous
"""Trainium2 Bass kernel for pairwise Tang-Toennies dispersion energy.

Problem: for P=3.2M random atom pairs over N=100k atoms in a periodic box,
    ene[p] = -(c6[ti,tj] * f6(b[ti,tj]*r)) / r^6   if r <= cutoff else 0
with r the minimum-image distance and f6 the 6th-order Tang-Toennies damper.

Only ~2% of random pairs fall inside the 10A cutoff, so the kernel is split
into a cheap dense screen and an exact sparse evaluation:

  host:     normalize coords to box units (x/L, an O(N) table prep; the
            i-side planes are negated so the device subtract is an add) and
            gather per-pair SoA planes jx..jz / ix..iz (pure index routing).
  kernel A: (dense, 3.2M slots) minimum-image r^2 per axis via
            m = min(|d|, 1-|d|) (exactly equal to the reference's round()
            form for |d| < 1), then a u8 in-cutoff mask.  Abs/Square run on
            the Act engine from a single activation-table set, so the hot
            loop has no table switches.
  host:     compacts the device-produced mask (np.flatnonzero + index
            gathers -- no host float math decides anything).
  kernel B: (sparse, ~62k slots) full energy: r^2 recomputed identically,
            r and r^-6 via Ln/Exp, Tang-Toennies f6 via an Estrin-form
            polynomial, exact series fallback for r^2<=0.4 where the
            1 - e^-u*poly form is pure f32 cancellation noise.
  host:     scatters the survivor energies into the zero-filled output.

Work is sharded by slots: every core gets the same slot count for both
kernels, so the 8 cores are perfectly balanced.
"""

import contextlib

import numpy as np

import concourse.bacc as bacc
import concourse.bass as bass
import concourse.mybir as mybir
from concourse.tile import TileContext
from concourse.bass_utils import run_bass_kernel_spmd

F32 = mybir.dt.float32
U8 = mybir.dt.uint8
U16 = mybir.dt.uint16
AF = mybir.ActivationFunctionType
OP = mybir.AluOpType

N_CORES = 8

# kernel A (dense screen) tiling: slots/core = 128 * KA * TA
KA = 1042
TA = 3
SLOTS_A = 128 * KA * TA          # 400,128 per core

# kernel B (sparse energy) tiling
KB = 80
TB = 1
SLOTS_B = 128 * KB * TB          # 10,240 per core (81,920 total capacity)

AXES = ("x", "y", "z")


def _geometry(nc, pw, w, K, Ls, nchunk, period=1.0, sub=False):
    """Fused 3-axis minimum-image: L^2 m^2 summed into r^2.

    `w` is a packed f32 tile [128, nchunk*K] whose chunks are
    [jx, ix, jy, iy, jz, iz, ...].  With sub=False the i chunks are
    NEGATED host-side (d = j + i_neg); with sub=True d = j - i.  Coords
    are in units of `period` per box length (1.0 for normalized f32,
    65536.0 for u16-quantized), folded into the min-image reflection and
    the Square scale.  All per-axis steps run as single strided ops over
    [128, 3, K].
    """
    jv = bass.AP(w.tensor, 0, [w[:].ap[0], [2 * K, 3], [1, K]])
    iv = bass.AP(w.tensor, K, [w[:].ap[0], [2 * K, 3], [1, K]])
    d = pw.tile([128, 3 * K], F32, tag="d", name="d")
    d3 = d[:].rearrange("p (a k) -> p a k", a=3)
    if sub:
        nc.vector.tensor_tensor(out=d3, in0=jv, in1=iv, op=OP.subtract)
    else:
        nc.gpsimd.tensor_tensor(out=d3, in0=jv, in1=iv, op=OP.add)
    a1 = pw.tile([128, 3 * K], F32, tag="a1", name="a1")
    nc.scalar.activation(a1[:], d[:], AF.Abs)
    w1 = pw.tile([128, 3 * K], F32, tag="w1", name="w1")
    nc.gpsimd.tensor_scalar(out=w1[:], in0=a1[:], scalar1=-1.0,
                            scalar2=float(period), op0=OP.mult, op1=OP.add)
    m = pw.tile([128, 3 * K], F32, tag="m", name="m")
    nc.vector.tensor_tensor(out=m[:], in0=a1[:], in1=w1[:], op=OP.min)
    sq = pw.tile([128, 3 * K], F32, tag="sq", name="sq")
    m3 = m[:].rearrange("p (a k) -> p a k", a=3)
    sq3 = sq[:].rearrange("p (a k) -> p a k", a=3)
    if Ls[0] == Ls[1] == Ls[2]:
        nc.scalar.activation(sq[:], m[:], AF.Square,
                             scale=float(Ls[0]) / float(period))
    else:
        for ci in range(3):
            nc.scalar.activation(sq3[:, ci, :], m3[:, ci, :], AF.Square,
                                 scale=float(Ls[ci]) / float(period))
    r2 = pw.tile([128, K], F32, tag="r2", name="r2")
    nc.vector.tensor_tensor(out=r2[:], in0=sq3[:, 0, :], in1=sq3[:, 1, :],
                            op=OP.add)
    nc.gpsimd.tensor_tensor(out=r2[:], in0=r2[:], in1=sq3[:, 2, :], op=OP.add)
    return r2


def build_a(Ls, cutoff, reps=1, t_limit=None):
    """Dense screen: per-slot minimum-image r^2 -> u8 (r2 <= cutoff^2).

    Separate f32 coordinate planes (i negated host-side), per-axis
    geometry: the 6 small DMAs spread across queues and all ops are
    contiguous, which measures fastest on hardware.
    """
    nc = bacc.Bacc(trn_type="TRN2", target_bir_lowering=False)
    jp = {ax: nc.dram_tensor(f"j{ax}", [SLOTS_A], F32, kind="ExternalInput")
          for ax in AXES}
    ip = {ax: nc.dram_tensor(f"i{ax}", [SLOTS_A], F32, kind="ExternalInput")
          for ax in AXES}
    mask_d = nc.dram_tensor("mask", [SLOTS_A], U8, kind="ExternalOutput")
    c2 = float(np.float32(cutoff) ** 2)
    T = TA if t_limit is None else min(TA, t_limit)

    with TileContext(nc) as tc:
        rep_ctx = tc.For_i(0, reps, 1) if reps > 1 else contextlib.nullcontext()
        with tc.tile_pool(name="io", bufs=2) as pio, \
             tc.tile_pool(name="work", bufs=2) as pw, rep_ctx:
            for t in range(T):
                jt, it = {}, {}
                for ax in AXES:
                    jt[ax] = pio.tile([128, KA], F32, tag=f"j{ax}",
                                      name=f"jt{ax}")
                    nc.sync.dma_start(
                        out=jt[ax][:],
                        in_=jp[ax][bass.ts(t, 128 * KA)].rearrange(
                            "(p x) -> p x", x=KA))
                    it[ax] = pio.tile([128, KA], F32, tag=f"i{ax}",
                                      name=f"it{ax}")
                    nc.sync.dma_start(
                        out=it[ax][:],
                        in_=ip[ax][bass.ts(t, 128 * KA)].rearrange(
                            "(p x) -> p x", x=KA))
                sq = []
                for ci, ax in enumerate(AXES):
                    d = pw.tile([128, KA], F32, tag=f"d{ax}", name=f"d{ax}")
                    nc.gpsimd.tensor_tensor(out=d[:], in0=jt[ax][:],
                                            in1=it[ax][:], op=OP.add)
                    a1 = pw.tile([128, KA], F32, tag=f"a{ax}", name=f"a1{ax}")
                    nc.scalar.activation(a1[:], d[:], AF.Abs)
                    w1 = pw.tile([128, KA], F32, tag=f"w{ax}", name=f"w1{ax}")
                    nc.vector.tensor_scalar(out=w1[:], in0=a1[:], scalar1=-1.0,
                                            scalar2=1.0, op0=OP.mult,
                                            op1=OP.add)
                    m = pw.tile([128, KA], F32, tag=f"m{ax}", name=f"m{ax}")
                    nc.vector.tensor_tensor(out=m[:], in0=a1[:], in1=w1[:],
                                            op=OP.min)
                    s = pw.tile([128, KA], F32, tag=f"s{ax}", name=f"s{ax}")
                    nc.scalar.activation(s[:], m[:], AF.Square,
                                         scale=float(Ls[ci]))
                    sq.append(s)
                r2 = pw.tile([128, KA], F32, tag="r2", name="r2")
                nc.vector.tensor_tensor(out=r2[:], in0=sq[0][:], in1=sq[1][:],
                                        op=OP.add)
                nc.gpsimd.tensor_tensor(out=r2[:], in0=r2[:], in1=sq[2][:],
                                        op=OP.add)
                mk = pio.tile([128, KA], U8, tag="mk")
                nc.vector.tensor_scalar(out=mk[:], in0=r2[:], scalar1=c2,
                                        scalar2=None, op0=OP.is_le)
                nc.sync.dma_start(
                    out=mask_d[bass.ts(t, 128 * KA)].rearrange(
                        "(p x) -> p x", x=KA),
                    in_=mk[:])
    nc.compile()
    return nc


def build_b(Ls, cutoff, reps=1):
    """Sparse exact energy on compacted in-cutoff slots.

    Inputs: negated-i coord planes, cp = -c6[ti,tj], bp = b[ti,tj].
    """
    nc = bacc.Bacc(trn_type="TRN2", target_bir_lowering=False)
    pk_d = nc.dram_tensor("pk", [TB * 128 * 8 * KB], F32, kind="ExternalInput")
    ene_d = nc.dram_tensor("ene", [SLOTS_B], F32, kind="ExternalOutput")
    c2 = float(np.float32(cutoff) ** 2)
    K = KB

    with TileContext(nc) as tc:
        rep_ctx = tc.For_i(0, reps, 1) if reps > 1 else contextlib.nullcontext()
        with tc.tile_pool(name="io", bufs=2) as pio, \
             tc.tile_pool(name="work", bufs=2) as pw, rep_ctx:
            for t in range(TB):
                w = pio.tile([128, 8 * K], F32, tag="w", name="w")
                nc.sync.dma_start(
                    out=w[:],
                    in_=pk_d[bass.ts(t, 128 * 8 * K)].rearrange(
                        "(p x) -> p x", x=8 * K))
                wv = w[:].rearrange("p (c k) -> p c k", c=8)
                cp = wv[:, 6, :]
                bp = wv[:, 7, :]

                # ---- geometry: identical arithmetic to kernel A ----
                r2 = _geometry(nc, pw, w, K, Ls, 8)

                # ---- r, r^-6 via one Ln + two Exp ----
                lr2 = pw.tile([128, K], F32, tag="lr2")
                nc.scalar.activation(lr2[:], r2[:], AF.Ln)
                rr = pw.tile([128, K], F32, tag="rr")
                nc.scalar.activation(rr[:], lr2[:], AF.Exp, scale=0.5)
                ir6 = pw.tile([128, K], F32, tag="ir6")
                nc.scalar.activation(ir6[:], lr2[:], AF.Exp, scale=-3.0)

                u = pw.tile([128, K], F32, tag="u")
                nc.vector.tensor_tensor(out=u[:], in0=bp, in1=rr[:],
                                        op=OP.mult)
                em = pw.tile([128, K], F32, tag="em")
                nc.scalar.activation(em[:], u[:], AF.Exp, scale=-1.0)
                w = pw.tile([128, K], F32, tag="w")
                nc.scalar.activation(w[:], u[:], AF.Square)

                # ---- poly(u) = sum_0^6 u^k/k!, balanced Estrin:
                # p = (1+u + w*(1/2+u/6)) + w^2*((1/24 + u/120) + w/720)
                a_ = pw.tile([128, K], F32, tag="a_")
                nc.vector.tensor_scalar(out=a_[:], in0=u[:], scalar1=1.0 / 6.0,
                                        scalar2=0.5, op0=OP.mult, op1=OP.add)
                bq = pw.tile([128, K], F32, tag="bq")
                nc.gpsimd.tensor_scalar(out=bq[:], in0=u[:], scalar1=1.0 / 120.0,
                                        scalar2=1.0 / 24.0, op0=OP.mult,
                                        op1=OP.add)
                t1 = pw.tile([128, K], F32, tag="t1")
                nc.gpsimd.tensor_scalar(out=t1[:], in0=u[:], scalar1=1.0,
                                        scalar2=None, op0=OP.add)
                w2 = pw.tile([128, K], F32, tag="w2")
                nc.scalar.activation(w2[:], w[:], AF.Square)
                d2 = pw.tile([128, K], F32, tag="d2")
                nc.vector.scalar_tensor_tensor(out=d2[:], in0=w[:],
                                               scalar=1.0 / 720.0, in1=bq[:],
                                               op0=OP.mult, op1=OP.add)
                g = pw.tile([128, K], F32, tag="g")
                nc.vector.tensor_tensor(out=g[:], in0=w[:], in1=a_[:],
                                        op=OP.mult)
                nc.vector.tensor_tensor(out=g[:], in0=g[:], in1=t1[:],
                                        op=OP.add)
                h = pw.tile([128, K], F32, tag="h")
                nc.vector.tensor_tensor(out=h[:], in0=w2[:], in1=d2[:],
                                        op=OP.mult)
                p = pw.tile([128, K], F32, tag="p")
                nc.vector.tensor_tensor(out=p[:], in0=g[:], in1=h[:],
                                        op=OP.add)

                # ---- ene = B'*poly - A' with A' = c6/r^6.  cp = -c6, so
                # An = -A', Bn = An*em = -B', ene = (-1*Bn)*poly + An. ----
                An = pw.tile([128, K], F32, tag="An")
                nc.vector.tensor_tensor(out=An[:], in0=cp, in1=ir6[:],
                                        op=OP.mult)
                Bn = pw.tile([128, K], F32, tag="Bn")
                nc.vector.tensor_tensor(out=Bn[:], in0=An[:], in1=em[:],
                                        op=OP.mult)
                ene = pw.tile([128, K], F32, tag="ene")
                nc.vector.scalar_tensor_tensor(out=ene[:], in0=Bn[:],
                                               scalar=-1.0, in1=p[:],
                                               op0=OP.mult, op1=OP.mult)
                nc.vector.tensor_tensor(out=ene[:], in0=ene[:], in1=An[:],
                                        op=OP.add)

                # ---- small-u exact series: f6 = em*u^7/5040*(1+u/8+u^2/72)
                # (the direct 1-em*poly form is f32 cancellation noise there;
                # es = (Bn/5040)*u7*S = -B'/5040*u7*S, correctly negative)
                u3 = pw.tile([128, K], F32, tag="u3")
                nc.gpsimd.tensor_tensor(out=u3[:], in0=u[:], in1=w[:],
                                        op=OP.mult)
                u6 = pw.tile([128, K], F32, tag="u6")
                nc.scalar.activation(u6[:], u3[:], AF.Square)
                u7 = pw.tile([128, K], F32, tag="u7")
                nc.gpsimd.tensor_tensor(out=u7[:], in0=u6[:], in1=u[:],
                                        op=OP.mult)
                s1u = pw.tile([128, K], F32, tag="s1u")
                nc.gpsimd.tensor_scalar(out=s1u[:], in0=u[:], scalar1=1.0 / 8.0,
                                        scalar2=1.0, op0=OP.mult, op1=OP.add)
                S = pw.tile([128, K], F32, tag="S")
                nc.vector.scalar_tensor_tensor(out=S[:], in0=w[:],
                                               scalar=1.0 / 72.0, in1=s1u[:],
                                               op0=OP.mult, op1=OP.add)
                es = pw.tile([128, K], F32, tag="es")
                nc.gpsimd.tensor_tensor(out=es[:], in0=u7[:], in1=S[:],
                                        op=OP.mult)
                nc.vector.scalar_tensor_tensor(out=es[:], in0=Bn[:],
                                               scalar=1.0 / 5040.0, in1=es[:],
                                               op0=OP.mult, op1=OP.mult)
                mu = pw.tile([128, K], U8, tag="mu")
                nc.gpsimd.tensor_scalar(out=mu[:], in0=r2[:], scalar1=0.4,
                                        scalar2=None, op0=OP.is_le)
                nc.vector.select(out=ene[:], mask=mu[:], on_true=es[:],
                                 on_false=ene[:])

                # ---- cutoff (identical compare to kernel A's mask) ----
                eout = pio.tile([128, K], F32, tag="eout")
                nc.vector.scalar_tensor_tensor(out=eout[:], in0=r2[:],
                                               scalar=c2, in1=ene[:],
                                               op0=OP.is_le, op1=OP.mult)
                nc.sync.dma_start(
                    out=ene_d[bass.ts(t, 128 * K)].rearrange(
                        "(p x) -> p x", x=K),
                    in_=eout[:])
    nc.compile()
    return nc


_NC_CACHE = {}


def _get_nc(builder, key, *args, **kw):
    if key not in _NC_CACHE:
        _NC_CACHE[key] = builder(*args, **kw)
    return _NC_CACHE[key]


def _host_reference(coords, pairs, box, c6, b, cutoff, atom_types):
    # numpy fallback for non-orthorhombic boxes (not hit by the real inputs)
    dr = coords[pairs[:, 1]] - coords[pairs[:, 0]]
    inv_box = np.linalg.inv(box)
    dr = dr - np.round(dr @ inv_box) @ box
    r = np.sqrt((dr * dr).sum(1))
    ti = atom_types[pairs[:, 0]]
    tj = atom_types[pairs[:, 1]]
    u = b[ti, tj] * r
    poly = 1.0 + u * (1.0 + u / 2.0 * (1.0 + u / 3.0 * (1.0 + u / 4.0 *
                     (1.0 + u / 5.0 * (1.0 + u / 6.0)))))
    f6 = 1.0 - np.exp(-u) * poly
    ene = -(c6[ti, tj] * f6) / r ** 6
    return np.where(r <= cutoff, ene, 0.0).astype(np.float32)


def _plane_sets_a(coords_n, pi, pj):
    """Per-core kernel-A inputs (normalized f32 coord SoA planes).

    The i planes are NEGATED (device computes d = j + (-i)).
    """
    P = pi.shape[0]
    total = N_CORES * SLOTS_A
    planes = {}
    for ci, ax in enumerate(AXES):
        pj_pl = np.full(total, 0.25, np.float32)
        pi_pl = np.zeros(total, np.float32)
        pj_pl[:P] = coords_n[pj, ci]
        pi_pl[:P] = -coords_n[pi, ci]
        planes[f"j{ax}"] = pj_pl
        planes[f"i{ax}"] = pi_pl
    return [{k: v[c * SLOTS_A:(c + 1) * SLOTS_A] for k, v in planes.items()}
            for c in range(N_CORES)]


def _bufs_b(coords_n, c6, b, pi, pj, ti, tj, sl):
    """Kernel B packed input (chunks [jx,ix,jy,iy,jz,iz,cp,bp]) for one
    chunk of survivor indices `sl`.  cp = -c6, i chunks negated."""
    cap = N_CORES * SLOTS_B
    n = sl.shape[0]
    pk = np.empty((8, cap), np.float32)
    for ci in range(3):
        pk[2 * ci, n:] = 0.25
        pk[2 * ci, :n] = coords_n[pj[sl], ci]
        pk[2 * ci + 1, n:] = 0.0
        pk[2 * ci + 1, :n] = -coords_n[pi[sl], ci]
    pk[6, n:] = 0.0
    pk[6, :n] = -c6[ti, tj]
    pk[7, n:] = 1.0
    pk[7, :n] = b[ti, tj]
    pk = pk.reshape(8, N_CORES, TB, 128, KB).transpose(1, 2, 3, 0, 4)
    pk = np.ascontiguousarray(pk).reshape(N_CORES, -1)
    return {"pk": pk}


def kernel(coords, pairs, box, c6, b, cutoff, atom_types):
    coords = np.asarray(coords, np.float32)
    pairs = np.asarray(pairs)
    box = np.asarray(box, np.float32)
    c6 = np.asarray(c6, np.float32)
    b = np.asarray(b, np.float32)
    atom_types = np.asarray(atom_types).astype(np.int64)
    cutoff = float(np.asarray(cutoff))

    offdiag = box - np.diag(np.diag(box))
    if np.any(offdiag != 0.0) or pairs.shape[0] > N_CORES * SLOTS_A:
        return _host_reference(coords, pairs, box, c6, b, cutoff, atom_types)
    Ls = tuple(float(box[i, i]) for i in range(3))

    P = pairs.shape[0]
    pi = np.ascontiguousarray(pairs[:, 0]).astype(np.int64)
    pj = np.ascontiguousarray(pairs[:, 1]).astype(np.int64)
    coords_n = coords / np.asarray(Ls, np.float32)[None, :]

    # ---- kernel A: dense in-cutoff screen ----
    nc_a = _get_nc(build_a, ("a", Ls, round(cutoff, 6)), Ls, cutoff)
    in_a = _plane_sets_a(coords_n, pi, pj)
    res_a = run_bass_kernel_spmd(nc_a, in_a, core_ids=list(range(N_CORES)))
    mask = np.concatenate([res_a.results[c]["mask"] for c in range(N_CORES)])

    # ---- host: compact by the device-produced mask (index routing only) ----
    idx = np.flatnonzero(mask[:P])

    # ---- kernel B: exact energies for survivors (chunked if ever needed) ----
    nc_b = _get_nc(build_b, ("b", Ls, round(cutoff, 6)), Ls, cutoff)
    cap = N_CORES * SLOTS_B
    ene_s = np.empty(idx.shape[0], np.float32)
    for lo in range(0, max(idx.shape[0], 1), cap):
        sl = idx[lo:lo + cap]
        ti = atom_types[pi[sl]]
        tj = atom_types[pj[sl]]
        buf = _bufs_b(coords_n, c6, b, pi, pj, ti, tj, sl)
        in_b = [{"pk": buf["pk"][c]} for c in range(N_CORES)]
        res_b = run_bass_kernel_spmd(nc_b, in_b, core_ids=list(range(N_CORES)))
        ene_full = np.concatenate([res_b.results[c]["ene"]
                                   for c in range(N_CORES)])
        ene_s[lo:lo + sl.shape[0]] = ene_full[:sl.shape[0]]

    out = np.zeros(P, np.float32)
    out[idx] = ene_s
    return out
